# revision 4
# baseline (speedup 1.0000x reference)
"""BitLinear forward (ternary-quantized linear) on 8 Trainium2 NeuronCores.

Computes out = x @ (clip(round(w/0.5), -1, 1) * scale[:, None]).T
for x:[4,2048,4096] f32, w:[11008,4096] f32, scale:[11008] f32.

Strategy (column-parallel, per the spec sharding hint):
  - Shard weight/scale along out_f: core c gets rows [c*1376, (c+1)*1376).
  - Replicate x; each core computes out[:, c*1376:(c+1)*1376].
  - Host passes x and the weight shard TRANSPOSED (contraction dim in_f
    outermost) so every device DMA is a natural-layout load; the gather is
    a concatenate along the feature axis.

Device kernel (per core):
  - DMA wT shard f32, quantize on device to ternary*scale, cached in SBUF
    as fp16 (ternary values are exact in fp16; x is the only rounded input).
  - Stream x m-tiles (128 tokens), cast f32->fp16 on DVE.
  - PE: out-tile [128 tok x {512,512,352} outf] accumulated over 32 k-tiles
    in PSUM (fp32); fp16 matmul runs at 1 cycle/row (4x faster than fp32).
  - ACT copies PSUM->SBUF, DMA to DRAM.
"""

import os

import numpy as np

import concourse.bass as bass
import concourse.mybir as mybir
import concourse.tile as tile
from concourse import bacc
from concourse.bass_utils import run_bass_kernel_spmd

P = 128
IN_F = 4096
OUT_F = 11008
BATCH = 4
SEQ = 2048
TOKENS = BATCH * SEQ  # 8192
N_CORES = 8
NSH = OUT_F // N_CORES  # 1376 out features per core

MAGIC = None  # unused; quantization is sign(w) * (|w| > 0.25)


def _n_chunks(nsh):
    """Split the out_f shard into moving-operand chunks of <=512 (PSUM bank)."""
    chunks = []
    n0 = 0
    while n0 < nsh:
        nw = min(512, nsh - n0)
        chunks.append((n0, nw))
        n0 += nw
    return chunks


def build_program(in_f=IN_F, tokens=TOKENS, nsh=NSH):
    """Build + compile the per-core Bass program (same program on all cores)."""
    ko_n = in_f // P  # k-tiles
    mt_n = tokens // P  # m-tiles (token tiles)
    chunks = _n_chunks(nsh)
    # x f32 staging granularity: ko-quarters keep SBUF pressure low
    stage_ko = min(8, ko_n)

    nc = bacc.Bacc("TRN2", target_bir_lowering=False, debug=False)

    xT = nc.dram_tensor("xT", [in_f, tokens], mybir.dt.float32, kind="ExternalInput")
    wT = nc.dram_tensor("wT", [in_f, nsh], mybir.dt.float32, kind="ExternalInput")
    scale = nc.dram_tensor("scale", [nsh], mybir.dt.float32, kind="ExternalInput")
    out = nc.dram_tensor("out", [tokens, nsh], mybir.dt.float32, kind="ExternalOutput")

    xT_ap = xT.ap().rearrange("(ko p) t -> p ko t", p=P)  # [128, ko_n, tokens]
    wT_ap = wT.ap()
    out_ap = out.ap()

    f32 = mybir.dt.float32
    f16 = mybir.dt.float16
    Alu = mybir.AluOpType

    with tile.TileContext(nc) as tc:
        with (
            tc.tile_pool(name="const", bufs=1) as const,
            tc.tile_pool(name="wqp", bufs=1) as wqp,
            tc.tile_pool(name="wst", bufs=2) as wst_pool,
            tc.tile_pool(name="qtmp", bufs=2) as qtmp,
            tc.tile_pool(name="xst", bufs=2) as xst_pool,
            tc.tile_pool(name="x16", bufs=2) as x16_pool,
            tc.tile_pool(name="ot", bufs=2) as ot_pool,
            tc.tile_pool(name="psum", bufs=6, space="PSUM") as psum,
        ):
            # scale broadcast across partitions: [128, nsh]
            scale_bc = const.tile([P, nsh], f32)
            sc_ap = scale.ap()
            sc_bcast = bass.AP(
                tensor=sc_ap.tensor, offset=sc_ap.offset, ap=[[0, P], *sc_ap.ap]
            )
            nc.sync.dma_start(scale_bc, sc_bcast)

            # Prefetch the first x m-tile before the quant loop so the PE's
            # first accumulation chain isn't gated on staging.
            def stage_x(mt):
                m0 = mt * P
                x16 = x16_pool.tile([P, ko_n, P], f16, tag="x16")
                for q0 in range(0, ko_n, stage_ko):
                    xst = xst_pool.tile([P, stage_ko, P], f32, tag="xst")
                    nc.sync.dma_start(xst, xT_ap[:, q0 : q0 + stage_ko, m0 : m0 + P])
                    # ACT is otherwise idle; DVE is the quant/copyback engine
                    nc.scalar.copy(x16[:, q0 : q0 + stage_ko, :], xst)
                return x16

            x16_first = stage_x(0)

            # Quantize the full weight shard once, cache PURE TERNARY fp16
            # [128, ko, nsh]; scale is folded into the PSUM->SBUF copyback.
            # ternary quant == (w > 0.25) - (w < -0.25); boundary values
            # land on round-half-even zero exactly like jnp.round(w/0.5).
            # Split across DVE + GpSimd so production keeps up with the PE.
            wq = wqp.tile([P, ko_n, nsh], f16)
            for ko in range(ko_n):
                wst = wst_pool.tile([P, nsh], f32, tag="wst")
                nc.sync.dma_start(wst, wT_ap[ko * P : (ko + 1) * P, :])
                pos = qtmp.tile([P, nsh], f32, tag="pos")
                nc.vector.tensor_scalar(pos, wst, 0.25, None, Alu.is_gt)
                neg = qtmp.tile([P, nsh], f32, tag="neg")
                nc.gpsimd.tensor_scalar(neg, wst, -0.25, None, Alu.is_lt)
                nc.vector.tensor_tensor(wq[:, ko, :], pos, neg, Alu.subtract)

            # Main matmul loop over token tiles
            for mt in range(mt_n):
                x16 = x16_first if mt == 0 else stage_x(mt)
                m0 = mt * P
                ot = ot_pool.tile([P, nsh], f32)
                for n0, nw in chunks:
                    ps = psum.tile([P, 512], f32, tag="ps")
                    for ko in range(ko_n):
                        nc.tensor.matmul(
                            ps[:, :nw],
                            x16[:, ko, :],
                            wq[:, ko, n0 : n0 + nw],
                            start=(ko == 0),
                            stop=(ko == ko_n - 1),
                        )
                    # copyback with the out_f scale folded in (DVE reads PSUM)
                    nc.vector.tensor_tensor(
                        ot[:, n0 : n0 + nw],
                        ps[:, :nw],
                        scale_bc[:, n0 : n0 + nw],
                        Alu.mult,
                    )
                nc.sync.dma_start(out_ap[m0 : m0 + P, :], ot)

    nc.compile()
    return nc


_PROGRAM = None


def _get_program():
    global _PROGRAM
    if _PROGRAM is None:
        _PROGRAM = build_program()
    return _PROGRAM


def _patch_artifact_upload():
    """Tracing uploads the NEFF dir to a shared bucket; in this container that
    can fail (no credentials) - degrade to a local-path no-op."""
    import concourse.bass_utils as bu

    orig = bu.upload_artifacts

    def safe_upload(tmpdir):
        try:
            return orig(tmpdir)
        except Exception:
            return tmpdir

    bu.upload_artifacts = safe_upload


def kernel(x, weight, scale):
    x = np.asarray(x, dtype=np.float32)
    weight = np.asarray(weight, dtype=np.float32)
    scale = np.asarray(scale, dtype=np.float32)

    xT = np.ascontiguousarray(x.reshape(TOKENS, IN_F).T)  # [in_f, tokens]
    in_maps = []
    for c in range(N_CORES):
        wc = weight[c * NSH : (c + 1) * NSH]  # [nsh, in_f]
        in_maps.append(
            {
                "xT": xT,
                "wT": np.ascontiguousarray(wc.T),  # [in_f, nsh]
                "scale": np.ascontiguousarray(scale[c * NSH : (c + 1) * NSH]),
            }
        )

    nc = _get_program()
    trace = os.environ.get("BASS_TRACE", "") == "1"
    if trace:
        _patch_artifact_upload()
    res = run_bass_kernel_spmd(nc, in_maps, core_ids=list(range(N_CORES)), trace=trace)
    kernel.last_results = res

    out = np.concatenate([res.results[c]["out"] for c in range(N_CORES)], axis=1)
    return out.reshape(BATCH, SEQ, OUT_F)


kernel.last_results = None


# revision 6
# speedup vs baseline: 1.4030x; 1.4030x over previous
"""BitLinear forward (ternary-quantized linear) on 8 Trainium2 NeuronCores.

Computes out = x @ (clip(round(w/0.5), -1, 1) * scale[:, None]).T
for x:[4,2048,4096] f32, w:[11008,4096] f32, scale:[11008] f32.

Strategy (column-parallel, per the spec sharding hint):
  - Shard weight/scale along out_f: core c gets rows [c*1376, (c+1)*1376).
  - Replicate x; each core computes out[:, c*1376:(c+1)*1376].
  - Host passes x and the weight shard TRANSPOSED (contraction dim in_f
    outermost) so every device DMA is a natural-layout load; the gather is
    a concatenate along the feature axis.

Device kernel (per core):
  - DMA wT shard f32, quantize on device to ternary*scale, cached in SBUF
    as fp16 (ternary values are exact in fp16; x is the only rounded input).
  - Stream x m-tiles (128 tokens), cast f32->fp16 on DVE.
  - PE: out-tile [128 tok x {512,512,352} outf] accumulated over 32 k-tiles
    in PSUM (fp32); fp16 matmul runs at 1 cycle/row (4x faster than fp32).
  - ACT copies PSUM->SBUF, DMA to DRAM.
"""

import os

import numpy as np

import concourse.bass as bass
import concourse.mybir as mybir
import concourse.tile as tile
from concourse import bacc
from concourse.bass_utils import run_bass_kernel_spmd

P = 128
IN_F = 4096
OUT_F = 11008
BATCH = 4
SEQ = 2048
TOKENS = BATCH * SEQ  # 8192
N_CORES = 8
NSH = OUT_F // N_CORES  # 1376 out features per core

MAGIC = None  # unused; quantization is sign(w) * (|w| > 0.25)


def _n_chunks(nsh):
    """Split the out_f shard into moving-operand chunks of <=512 (PSUM bank)."""
    chunks = []
    n0 = 0
    while n0 < nsh:
        nw = min(512, nsh - n0)
        chunks.append((n0, nw))
        n0 += nw
    return chunks


def build_program(in_f=IN_F, tokens=TOKENS, nsh=NSH):
    """Build + compile the per-core Bass program (same program on all cores)."""
    ko_n = in_f // P  # k-tiles
    mt_n = tokens // P  # m-tiles (token tiles)
    chunks = _n_chunks(nsh)
    # x f32 staging granularity: ko-quarters keep SBUF pressure low
    stage_ko = min(8, ko_n)

    # phase-1 token tiles processed chunk-major so the PE can start while the
    # weight-shard quantization is still streaming out of the DVE
    G = min(3, mt_n)

    nc = bacc.Bacc("TRN2", target_bir_lowering=False, debug=False)

    xT = nc.dram_tensor("xT", [in_f, tokens], mybir.dt.float32, kind="ExternalInput")
    wT = nc.dram_tensor("wT", [in_f, nsh], mybir.dt.float32, kind="ExternalInput")
    scale = nc.dram_tensor("scale", [nsh], mybir.dt.float32, kind="ExternalInput")
    out = nc.dram_tensor("out", [tokens, nsh], mybir.dt.float32, kind="ExternalOutput")

    xT_ap = xT.ap().rearrange("(ko p) t -> p ko t", p=P)  # [128, ko_n, tokens]
    wT_ap = wT.ap()
    out_ap = out.ap()

    f32 = mybir.dt.float32
    f16 = mybir.dt.float16
    Alu = mybir.AluOpType

    with tile.TileContext(nc) as tc:
        with (
            tc.tile_pool(name="const", bufs=1) as const,
            tc.tile_pool(name="wqp", bufs=1) as wqp,
            tc.tile_pool(name="wst", bufs=3) as wst_pool,
            tc.tile_pool(name="qtmp", bufs=2) as qtmp,
            tc.tile_pool(name="xst", bufs=2) as xst_pool,
            tc.tile_pool(name="x16", bufs=4) as x16_pool,
            tc.tile_pool(name="otc", bufs=4) as otc_pool,
            tc.tile_pool(name="psum", bufs=6, space="PSUM") as psum,
        ):
            # scale broadcast across partitions: [128, nsh]
            scale_bc = const.tile([P, nsh], f32)
            sc_ap = scale.ap()
            sc_bcast = bass.AP(
                tensor=sc_ap.tensor, offset=sc_ap.offset, ap=[[0, P], *sc_ap.ap]
            )
            nc.sync.dma_start(scale_bc, sc_bcast)

            def stage_x(mt):
                m0 = mt * P
                x16 = x16_pool.tile([P, ko_n, P], f16, tag="x16")
                for q0 in range(0, ko_n, stage_ko):
                    xst = xst_pool.tile([P, stage_ko, P], f32, tag="xst")
                    nc.sync.dma_start(xst, xT_ap[:, q0 : q0 + stage_ko, m0 : m0 + P])
                    # ACT engine; DVE is reserved for quantization/scale
                    nc.scalar.copy(x16[:, q0 : q0 + stage_ko, :], xst)
                return x16

            def chain(x16, mt, n0, nw):
                """One PSUM accumulation chain + copyback + scale + store."""
                m0 = mt * P
                ps = psum.tile([P, 512], f32, tag="ps")
                for ko in range(ko_n):
                    nc.tensor.matmul(
                        ps[:, :nw],
                        x16[:, ko, :],
                        wq[:, ko, n0 : n0 + nw],
                        start=(ko == 0),
                        stop=(ko == ko_n - 1),
                    )
                otc = otc_pool.tile([P, 512], f32, tag="otc")
                nc.scalar.copy(otc[:, :nw], ps[:, :nw])  # ACT reads PSUM fast
                nc.vector.tensor_tensor(
                    otc[:, :nw], otc[:, :nw], scale_bc[:, n0 : n0 + nw], Alu.mult
                )
                nc.sync.dma_start(out_ap[m0 : m0 + P, n0 : n0 + nw], otc[:, :nw])

            # Prefetch phase-1 x tiles up front (ACT), so the PE's first chains
            # aren't gated on staging.
            x16s = {mt: stage_x(mt) for mt in range(G)}

            # Quantize the weight shard to PURE TERNARY fp16 [128, ko, nsh],
            # chunk-major so the PE can consume chunk 0 while later chunks are
            # still quantizing. scale is applied to the output tiles instead.
            # ternary quant == (w > 0.25) - (w < -0.25); boundary values land
            # on round-half-even zero exactly like jnp.round(w/0.5).
            wq = wqp.tile([P, ko_n, nsh], f16)
            for n0, nw in chunks:
                for ko in range(ko_n):
                    wst = wst_pool.tile([P, 512], f32, tag="wst")
                    nc.sync.dma_start(
                        wst[:, :nw], wT_ap[ko * P : (ko + 1) * P, n0 : n0 + nw]
                    )
                    pos = qtmp.tile([P, 512], f32, tag="pos")
                    nc.vector.tensor_scalar(pos[:, :nw], wst[:, :nw], 0.25, None, Alu.is_gt)
                    neg = qtmp.tile([P, 512], f32, tag="neg")
                    nc.vector.tensor_scalar(neg[:, :nw], wst[:, :nw], -0.25, None, Alu.is_lt)
                    nc.vector.tensor_tensor(
                        wq[:, ko, n0 : n0 + nw], pos[:, :nw], neg[:, :nw], Alu.subtract
                    )

            # Phase 1: chunk-major over the first G token tiles (rides the
            # quantization as it streams out chunk by chunk).
            for n0, nw in chunks:
                for mt in range(G):
                    chain(x16s[mt], mt, n0, nw)

            # Steady state: token-tile-major.
            for mt in range(G, mt_n):
                x16 = stage_x(mt)
                for n0, nw in chunks:
                    chain(x16, mt, n0, nw)

    nc.compile()
    return nc


_PROGRAM = None


def _get_program():
    global _PROGRAM
    if _PROGRAM is None:
        _PROGRAM = build_program()
    return _PROGRAM


def _patch_artifact_upload():
    """Tracing uploads the NEFF dir to a shared bucket; in this container that
    can fail (no credentials) - degrade to a local-path no-op."""
    import concourse.bass_utils as bu

    orig = bu.upload_artifacts

    def safe_upload(tmpdir):
        try:
            return orig(tmpdir)
        except Exception:
            return tmpdir

    bu.upload_artifacts = safe_upload


def kernel(x, weight, scale):
    x = np.asarray(x, dtype=np.float32)
    weight = np.asarray(weight, dtype=np.float32)
    scale = np.asarray(scale, dtype=np.float32)

    xT = np.ascontiguousarray(x.reshape(TOKENS, IN_F).T)  # [in_f, tokens]
    in_maps = []
    for c in range(N_CORES):
        wc = weight[c * NSH : (c + 1) * NSH]  # [nsh, in_f]
        in_maps.append(
            {
                "xT": xT,
                "wT": np.ascontiguousarray(wc.T),  # [in_f, nsh]
                "scale": np.ascontiguousarray(scale[c * NSH : (c + 1) * NSH]),
            }
        )

    nc = _get_program()
    trace = os.environ.get("BASS_TRACE", "") == "1"
    if trace:
        _patch_artifact_upload()
    res = run_bass_kernel_spmd(nc, in_maps, core_ids=list(range(N_CORES)), trace=trace)
    kernel.last_results = res

    out = np.concatenate([res.results[c]["out"] for c in range(N_CORES)], axis=1)
    return out.reshape(BATCH, SEQ, OUT_F)


kernel.last_results = None


# revision 11
# speedup vs baseline: 1.4215x; 1.0132x over previous
"""BitLinear forward (ternary-quantized linear) on 8 Trainium2 NeuronCores.

Computes out = x @ (clip(round(w/0.5), -1, 1) * scale[:, None]).T
for x:[4,2048,4096] f32, w:[11008,4096] f32, scale:[11008] f32.

Strategy (column-parallel, per the spec sharding hint):
  - Shard weight/scale along out_f: core c gets rows [c*1376, (c+1)*1376).
  - Replicate x; each core computes out[:, c*1376:(c+1)*1376].
  - Host passes x and the weight shard TRANSPOSED (contraction dim in_f
    outermost) so every device DMA is a natural-layout load; the gather is
    a concatenate along the feature axis.

Device kernel (per core):
  - DMA wT shard f32, quantize on device to ternary*scale, cached in SBUF
    as fp16 (ternary values are exact in fp16; x is the only rounded input).
  - Stream x m-tiles (128 tokens), cast f32->fp16 on DVE.
  - PE: out-tile [128 tok x {512,512,352} outf] accumulated over 32 k-tiles
    in PSUM (fp32); fp16 matmul runs at 1 cycle/row (4x faster than fp32).
  - ACT copies PSUM->SBUF, DMA to DRAM.
"""

import os

import numpy as np

import concourse.bass as bass
import concourse.mybir as mybir
import concourse.tile as tile
from concourse import bacc
from concourse.bass_utils import run_bass_kernel_spmd

P = 128
IN_F = 4096
OUT_F = 11008
BATCH = 4
SEQ = 2048
TOKENS = BATCH * SEQ  # 8192
N_CORES = 8
NSH = OUT_F // N_CORES  # 1376 out features per core

MAGIC = None  # unused; quantization is sign(w) * (|w| > 0.25)


def _n_chunks(nsh):
    """Split the out_f shard into moving-operand chunks of <=512 (PSUM bank)."""
    chunks = []
    n0 = 0
    while n0 < nsh:
        nw = min(512, nsh - n0)
        chunks.append((n0, nw))
        n0 += nw
    return chunks


def build_program(in_f=IN_F, tokens=TOKENS, nsh=NSH):
    """Build + compile the per-core Bass program (same program on all cores)."""
    ko_n = in_f // P  # k-tiles
    mt_n = tokens // P  # m-tiles (token tiles)
    chunks = _n_chunks(nsh)
    # x f32 staging granularity: ko-quarters keep SBUF pressure low
    stage_ko = min(8, ko_n)

    # phase-1 token tiles processed chunk-major, their accumulation chains
    # interleaved ko-outer in lockstep, so the PE stays busy while the
    # weight-shard quantization is still streaming out of the DVE
    G = min(4, mt_n)

    nc = bacc.Bacc("TRN2", target_bir_lowering=False, debug=False)

    xT = nc.dram_tensor("xT", [in_f, tokens], mybir.dt.float32, kind="ExternalInput")
    wT = nc.dram_tensor("wT", [in_f, nsh], mybir.dt.float32, kind="ExternalInput")
    scale = nc.dram_tensor("scale", [nsh], mybir.dt.float32, kind="ExternalInput")
    out = nc.dram_tensor("out", [tokens, nsh], mybir.dt.float32, kind="ExternalOutput")

    xT_ap = xT.ap().rearrange("(ko p) t -> p ko t", p=P)  # [128, ko_n, tokens]
    wT_ap = wT.ap()
    out_ap = out.ap()

    f32 = mybir.dt.float32
    f16 = mybir.dt.float16
    Alu = mybir.AluOpType

    with tile.TileContext(nc) as tc:
        with (
            tc.tile_pool(name="const", bufs=1) as const,
            tc.tile_pool(name="wqp", bufs=1) as wqp,
            tc.tile_pool(name="wst", bufs=3) as wst_pool,
            tc.tile_pool(name="qtmp", bufs=2) as qtmp,
            tc.tile_pool(name="xst", bufs=2) as xst_pool,
            tc.tile_pool(name="x16", bufs=G + 1) as x16_pool,
            tc.tile_pool(name="otc", bufs=4) as otc_pool,
            tc.tile_pool(name="psum", bufs=8, space="PSUM") as psum,
        ):
            # scale broadcast across partitions: [128, nsh]
            scale_bc = const.tile([P, nsh], f32)
            sc_ap = scale.ap()
            sc_bcast = bass.AP(
                tensor=sc_ap.tensor, offset=sc_ap.offset, ap=[[0, P], *sc_ap.ap]
            )
            nc.sync.dma_start(scale_bc, sc_bcast)

            def stage_x(mt):
                m0 = mt * P
                x16 = x16_pool.tile([P, ko_n, P], f16, tag="x16")
                for q0 in range(0, ko_n, stage_ko):
                    xst = xst_pool.tile([P, stage_ko, P], f32, tag="xst")
                    nc.sync.dma_start(xst, xT_ap[:, q0 : q0 + stage_ko, m0 : m0 + P])
                    # ACT engine; DVE is reserved for quantization/scale
                    nc.scalar.copy(x16[:, q0 : q0 + stage_ko, :], xst)
                return x16

            def finish_chain(ps, mt, n0, nw):
                """Copyback + scale + store for one finished PSUM chain."""
                m0 = mt * P
                otc = otc_pool.tile([P, 512], f32, tag="otc")
                nc.scalar.copy(otc[:, :nw], ps[:, :nw])  # ACT reads PSUM fast
                nc.vector.tensor_tensor(
                    otc[:, :nw], otc[:, :nw], scale_bc[:, n0 : n0 + nw], Alu.mult
                )
                nc.sync.dma_start(out_ap[m0 : m0 + P, n0 : n0 + nw], otc[:, :nw])

            def chain(x16, mt, n0, nw):
                """One PSUM accumulation chain + copyback + scale + store."""
                ps = psum.tile([P, 512], f32, tag="ps")
                for ko in range(ko_n):
                    nc.tensor.matmul(
                        ps[:, :nw],
                        x16[:, ko, :],
                        wq[:, ko, n0 : n0 + nw],
                        start=(ko == 0),
                        stop=(ko == ko_n - 1),
                    )
                finish_chain(ps, mt, n0, nw)

            # Prefetch phase-1 x tiles up front (ACT), so the PE's first chains
            # aren't gated on staging.
            x16s = {mt: stage_x(mt) for mt in range(G)}

            # Quantize the weight shard to PURE TERNARY fp16 [128, ko, nsh],
            # chunk-major so the PE can consume chunk 0 while later chunks are
            # still quantizing. scale is applied to the output tiles instead.
            # ternary quant == (w > 0.25) - (w < -0.25); boundary values land
            # on round-half-even zero exactly like jnp.round(w/0.5).
            wq = wqp.tile([P, ko_n, nsh], f16)
            for n0, nw in chunks:
                for ko in range(ko_n):
                    wst = wst_pool.tile([P, 512], f32, tag="wst")
                    nc.sync.dma_start(
                        wst[:, :nw], wT_ap[ko * P : (ko + 1) * P, n0 : n0 + nw]
                    )
                    pos = qtmp.tile([P, 512], f32, tag="pos")
                    nc.vector.tensor_scalar(pos[:, :nw], wst[:, :nw], 0.25, None, Alu.is_gt)
                    neg = qtmp.tile([P, 512], f32, tag="neg")
                    nc.vector.tensor_scalar(neg[:, :nw], wst[:, :nw], -0.25, None, Alu.is_lt)
                    nc.vector.tensor_tensor(
                        wq[:, ko, n0 : n0 + nw], pos[:, :nw], neg[:, :nw], Alu.subtract
                    )

            # Phase 1: chunk-major over the first G token tiles, the G chains
            # interleaved ko-outer in lockstep — the PE issues G matmuls per
            # quantized k-subtile, so it keeps pace with the DVE production.
            for n0, nw in chunks:
                pss = [
                    psum.tile([P, 512], f32, tag="ps", name=f"ps_p1_{g}")
                    for g in range(G)
                ]
                for ko in range(ko_n):
                    for g in range(G):
                        nc.tensor.matmul(
                            pss[g][:, :nw],
                            x16s[g][:, ko, :],
                            wq[:, ko, n0 : n0 + nw],
                            start=(ko == 0),
                            stop=(ko == ko_n - 1),
                        )
                for g in range(G):
                    finish_chain(pss[g], g, n0, nw)

            # Steady state: token-tile-major.
            for mt in range(G, mt_n):
                x16 = stage_x(mt)
                for n0, nw in chunks:
                    chain(x16, mt, n0, nw)

    nc.compile()
    return nc


_PROGRAM = None


def _get_program():
    global _PROGRAM
    if _PROGRAM is None:
        _PROGRAM = build_program()
    return _PROGRAM


def _patch_artifact_upload():
    """Tracing uploads the NEFF dir to a shared bucket; in this container that
    can fail (no credentials) - degrade to a local-path no-op."""
    import concourse.bass_utils as bu

    orig = bu.upload_artifacts

    def safe_upload(tmpdir):
        try:
            return orig(tmpdir)
        except Exception:
            return tmpdir

    bu.upload_artifacts = safe_upload


def kernel(x, weight, scale):
    x = np.asarray(x, dtype=np.float32)
    weight = np.asarray(weight, dtype=np.float32)
    scale = np.asarray(scale, dtype=np.float32)

    xT = np.ascontiguousarray(x.reshape(TOKENS, IN_F).T)  # [in_f, tokens]
    in_maps = []
    for c in range(N_CORES):
        wc = weight[c * NSH : (c + 1) * NSH]  # [nsh, in_f]
        in_maps.append(
            {
                "xT": xT,
                "wT": np.ascontiguousarray(wc.T),  # [in_f, nsh]
                "scale": np.ascontiguousarray(scale[c * NSH : (c + 1) * NSH]),
            }
        )

    nc = _get_program()
    trace = os.environ.get("BASS_TRACE", "") == "1"
    if trace:
        _patch_artifact_upload()
    res = run_bass_kernel_spmd(nc, in_maps, core_ids=list(range(N_CORES)), trace=trace)
    kernel.last_results = res

    out = np.concatenate([res.results[c]["out"] for c in range(N_CORES)], axis=1)
    return out.reshape(BATCH, SEQ, OUT_F)


kernel.last_results = None
